# revision 73
# baseline (speedup 1.0000x reference)
"""Trainium2 Bass kernel for LongcatFlashMLA prefill (B=2, L=2048, H=16 MLA).

Sharding: core c handles batch c//4 and heads 4*(c%4) .. 4*(c%4)+4 (tensor
parallel over heads, data parallel over batch). Each core computes a partial
output [L, HID] (its heads' contribution through o_proj); host sums the 4
partials per batch.

Device-side layout is feature-major ("T" = [feature, seq]); all transposes
are done on host. Structural optimizations vs the straightforward pipeline:

- q path is algebraically folded: q = rmsnorm(x@wq_a.T)@wq_b.T
  = (x @ (wq_b@wq_a).T) * (1/rms per token), so the q_b stage disappears.
  The 1/rms needs ||x@wq_a.T||^2 per token only, which tolerates heavy
  quantization (errors average over QL=1536 squares): that pass runs in
  fp8e4m3 with DoubleRow perf mode (2 K-tiles per pass at 0.5 cyc/col, 4x
  the bf16 PE rate). Everything value-carrying stays bf16: fp8's ~3%
  per-element noise does NOT average out through attention (the signal
  p@v of near-random v shrinks by the same sqrt(n_eff) as the noise).
- scores are computed TRANSPOSED (S^T = K Q^T, k on partitions): the exp
  output is directly P^T, which is exactly the lhsT layout attn@V needs,
  eliminating the per-block P transposes (and their DVE copies) of a
  q-major layout.
- V carries an extra ones column so the softmax row-sum falls out of the
  attn@V matmul itself; the 1/rowsum is applied as a per-partition scale
  on the tiny [q, VD] eviction.
"""

import math
import sys
from contextlib import ExitStack

import numpy as np

if "/opt/trn_rl_repo" not in sys.path:
    sys.path.insert(0, "/opt/trn_rl_repo")

import ml_dtypes

B, L, HID = 2, 2048, 2048
H = 16
QL = 1536
KVL = 512
ROPE = 64
NOPE = 128
VD = 128
QKD = NOPE + ROPE
SCALE = QKD ** -0.5
EPS = 1e-5
THETA = 10000.0
SQ = (HID / QL) ** 0.5
SKV = (HID / KVL) ** 0.5

NCORES = 8
GPB = 4              # head-groups (cores) per batch
HPG = H // GPB       # heads per core
CH = 512             # L chunk processed per pipeline stage
NCH = L // CH
QT = CH // 128       # q-tiles (128 rows) per chunk
KT = HID // 128      # k-tiles of the HID contraction
KP = KT // 2         # DoubleRow k-tile pairs
MT = QL // 128       # row-tiles of qa (for sumsq)
QBT = (HPG * QKD) // 128   # row-tiles of folded q output (768 rows -> 6)
KVT = KVL // 128     # 4
NBLK = L // 128      # 16 kpos blocks total
VB = VD + 2          # v block stride (col VD holds ones, 1 pad)

S_A = 6              # wq_a fp8 boost (sumsq path is scale-compensated)
EXP_BIAS = -2.0      # constant score shift; cancels in softmax normalization

BF16 = ml_dtypes.bfloat16
F8 = ml_dtypes.float8_e4m3

_PROG_CACHE = {}


def _bf(a):
    return np.ascontiguousarray(a.astype(np.float32)).astype(BF16)


def _f8(a):
    return np.ascontiguousarray(a.astype(np.float32)).astype(F8)


def _emit(ctx, tc, t, causal):
    import concourse.bass as bass
    from concourse import mybir

    nc = tc.nc
    f32 = mybir.dt.float32
    bf = mybir.dt.bfloat16
    f8 = mybir.dt.float8e4
    AF = mybir.ActivationFunctionType
    MULT = mybir.AluOpType.mult
    DR = mybir.MatmulPerfMode.DoubleRow

    const = ctx.enter_context(tc.tile_pool(name="const", bufs=1))
    w8stream = ctx.enter_context(tc.tile_pool(name="w8stream", bufs=5))
    xlo_p = ctx.enter_context(tc.tile_pool(name="xlo", bufs=1))
    x8_p = ctx.enter_context(tc.tile_pool(name="x8", bufs=2))
    ckvb_p = ctx.enter_context(tc.tile_pool(name="ckvb", bufs=2))
    qpe_p = ctx.enter_context(tc.tile_pool(name="qpe", bufs=1))
    sq_p = ctx.enter_context(tc.tile_pool(name="sq", bufs=2))
    sm_p = ctx.enter_context(tc.tile_pool(name="sm", bufs=3))
    rope_p = ctx.enter_context(tc.tile_pool(name="rope", bufs=2))
    rbp_p = ctx.enter_context(tc.tile_pool(name="rbp", bufs=2))
    persist = ctx.enter_context(tc.tile_pool(name="persist", bufs=1))
    psb_p = ctx.enter_context(tc.tile_pool(name="psb", bufs=1))
    pvb_p = ctx.enter_context(tc.tile_pool(name="pvb", bufs=3))
    oh_p = ctx.enter_context(tc.tile_pool(name="oh", bufs=1))
    st_p = ctx.enter_context(tc.tile_pool(name="stage", bufs=2))
    rec_p = ctx.enter_context(tc.tile_pool(name="rec", bufs=4))
    ssr_p = ctx.enter_context(tc.tile_pool(name="ssr", bufs=2))

    psA = ctx.enter_context(tc.tile_pool(name="psA", bufs=2, space="PSUM"))
    psS = ctx.enter_context(tc.tile_pool(name="psS", bufs=2, space="PSUM"))
    psPV = ctx.enter_context(tc.tile_pool(name="psPV", bufs=2, space="PSUM"))
    psT = ctx.enter_context(tc.tile_pool(name="psT", bufs=1, space="PSUM"))
    psO = ctx.enter_context(tc.tile_pool(name="psO", bufs=1, space="PSUM"))

    # ---- resident constants / weights (DMAs deferred to first use so the
    # first chunk's compute isn't stuck behind them)
    wemb_t = const.tile([128, KVT, HPG * NOPE], bf, name="wemb")
    wunb_t = const.tile([128, KVT, HPG * VD], bf, name="wunb")
    wot_t = const.tile([128, HPG, HID], f8, name="wot")
    wol_t = const.tile([128, HPG, HID], f8, name="wol")
    cos_t = const.tile([128, L], bf, name="cos")
    sin_t = const.tile([128, L], bf, name="sin")
    perm_t = const.tile([128, 128], bf, name="perm")
    dup_t = const.tile([64, 128], bf, name="dup")
    id_t = const.tile([128, 128], bf, name="ident")
    tri_t = const.tile([128, 128], f32, name="tri") if causal else None

    def load_consts_early():
        nc.sync.dma_start(out=cos_t[:], in_=t["cos"][:])
        nc.sync.dma_start(out=sin_t[:], in_=t["sin"][:])
        nc.sync.dma_start(out=perm_t[:], in_=t["perm"][:])
        nc.sync.dma_start(out=dup_t[:], in_=t["dup"][:])
        nc.sync.dma_start(out=wemb_t[:],
                          in_=t["wemb"][:])
        nc.sync.dma_start(out=wunb_t[:],
                          in_=t["wunb"][:])

    def load_consts_attn():
        nc.sync.dma_start(out=id_t[:], in_=t["ident"][:])
        if causal:
            nc.sync.dma_start(out=tri_t[:], in_=t["tri"][:])
        nc.sync.dma_start(out=wot_t[:], in_=t["wot"][:])
        nc.sync.dma_start(out=wol_t[:], in_=t["wol"][:])

    onec_t = const.tile([128, 1], bf, name="onec")
    nc.vector.memset(onec_t[:], 1.0)
    oner_t = const.tile([1, 128], bf, name="oner")
    nc.vector.memset(oner_t[:], 1.0)
    eps12_t = const.tile([128, 1], f32, name="eps12")
    nc.vector.memset(eps12_t[:], EPS * 4096.0)
    eps18_t = const.tile([128, 1], f32, name="eps18")
    nc.vector.memset(eps18_t[:], EPS * 262144.0)
    m2_t = const.tile([128, 1], f32, name="m2")
    nc.vector.memset(m2_t[:], EXP_BIAS)

    # ---- persistent caches
    kno_t = [persist.tile([128, L], bf, name=f"kno{h}") for h in range(HPG)]
    kpe_t = persist.tile([128, L], bf, name="kpe")  # roped k_pe, dup'd halves
    v_t = [persist.tile([128, NBLK, VB], bf, name=f"v{h}") for h in range(HPG)]
    for h in range(HPG):
        nc.vector.memset(v_t[h][:, :, VD:VD + 1], 0.0625)
    pre_rrb = persist.tile([1, NCH, CH], bf, name="prerrb")
    nq = NCH if not causal else 1
    qn_all = persist.tile([128, nq, HPG, CH], bf, name="qn") if not causal \
        else None
    qpe_all = persist.tile([128, nq, 2, CH], bf, name="qpe") if not causal \
        else None

    def rope_combine(dst, src, pp, cs):
        """dst = src*cos + perm(src)*sin ([128, CH], bf16)."""
        tmp = rope_p.tile([128, CH], bf, name="rtmp")
        nc.vector.tensor_tensor(tmp[:], pp[:], sin_t[:, cs], op=MULT)
        nc.vector.tensor_tensor(dst[:], src[:], cos_t[:, cs], op=MULT)
        nc.vector.tensor_add(dst[:], dst[:], tmp[:])

    def rms_scalar(pss, scale, eps_ap, name):
        """pss [1, CH] = sum of squares -> rrb [1, CH] bf16 = 1/rms-ish.
        Act/DVE only, so it drains behind the next loop's PE work."""
        rs = sm_p.tile([1, CH], f32, name=f"rs{name}")
        nc.scalar.activation(rs[:], pss[:1, :], AF.Sqrt, bias=eps_ap,
                             scale=scale)
        rr = sm_p.tile([1, CH], f32, name=f"rr{name}")
        nc.vector.reciprocal(rr[:], rs[:])
        rrb = sm_p.tile([1, CH], bf, name=f"rrb{name}")
        nc.vector.tensor_copy(rrb[:], rr[:])
        return rrb

    def rms_bcast(rrb, name):
        """Broadcast [1, CH] -> [128, CH] via ones-column matmul."""
        pb = psA.tile([128, CH], f32, name="pa")
        nc.tensor.matmul(pb[:], oner_t[:], rrb[:], start=True, stop=True)
        rbp = rbp_p.tile([128, CH], bf, name=f"rbp{name}")
        nc.scalar.copy(rbp[:], pb[:])
        return rbp

    def load_x(c):
        """DMA chunk c of x, pre-split on host into fp8 hi + residual."""
        x8 = x8_p.tile([128, KT, CH], f8, name="x8")
        xlo = xlo_p.tile([128, KT, CH], f8, name="xlo")
        if c == 0:
            # split so the first qa8 matmuls wait only on their k-tiles
            for s in range(4):
                nc.sync.dma_start(out=x8[:, 4 * s:4 * s + 4, :],
                                  in_=t["x8"][c, :, 4 * s:4 * s + 4, :])
        else:
            nc.sync.dma_start(out=x8[:], in_=t["x8"][c])
        nc.sync.dma_start(out=xlo[:], in_=t["xlo"][c])
        return x8, xlo

    def prefetch_w8(n=2):
        pre = []
        for m in range(n):
            wcol = w8stream.tile([128, KT, 128], f8, name="wqa")
            nc.sync.dma_start(out=wcol[:], in_=t["wqa"][m])
            pre.append(wcol)
        return pre

    def phases_proj(c, qn_t, qpe_t, x8, xlo, pre_w8=None):
        """Projections for chunk c: fills kno/kpe[:, cs], v blocks, qn/qpe."""
        cs = slice(c * CH, (c + 1) * CH)

        # --- qa8 sumsq: chunk 0 computes it fully locally; later chunks use
        # the sharded prepass result (one cross-core AllReduce, hidden
        # behind chunk 0's compute)
        if c == 0:
            pss_q = psS.tile([128, CH], f32, name="ps")
            for m in range(MT):
                if pre_w8 and m < len(pre_w8):
                    wcol = pre_w8[m]
                else:
                    wcol = w8stream.tile([128, KT, 128], f8, name="wqa")
                    nc.sync.dma_start(out=wcol[:], in_=t["wqa"][m])
                pa = psA.tile([128, CH], f32, name="pa")
                for k in range(KP):
                    nc.tensor.matmul(
                        pa[:], wcol[:, 2 * k:2 * k + 2, :],
                        x8[:, 2 * k:2 * k + 2, :],
                        start=(k == 0), stop=(k == KP - 1), perf_mode=DR,
                    )
                sqt = sq_p.tile([128, CH], bf, name="sq")
                nc.scalar.square(sqt[:], pa[:])
                nc.tensor.matmul(pss_q[:1, :], onec_t[:], sqt[:],
                                 start=(m == 0), stop=(m == MT - 1))
            rrb_q = rms_scalar(pss_q, 64.0 / QL, eps18_t[:1, :], "q")
        else:
            rrb_q = pre_rrb[:1, c, :]

        # --- kv_a (bf16) + squares of the latent part
        ckvb = ckvb_p.tile([128, KVT + 1, CH], bf, name="ckvb")
        pss_kv = psS.tile([128, CH], f32, name="ps")
        for m in range(KVT + 1):
            rows = 128 if m < KVT else ROPE
            whl = w8stream.tile([128, 2, KT, 128], f8, name="wqa")
            nc.sync.dma_start(out=whl[:], in_=t["wkva2"][m])
            if c == 0 and m == 0:
                load_consts_early()
            pa = psA.tile([128, CH], f32, name="pa")
            for pi, (wi, xp) in enumerate(((0, x8), (1, x8), (0, xlo))):
                for k in range(KP):
                    nc.tensor.matmul(
                        pa[:rows, :],
                        whl[:, wi, 2 * k:2 * k + 2, :rows],
                        xp[:, 2 * k:2 * k + 2, :],
                        start=(pi == 0 and k == 0),
                        stop=(pi == 2 and k == KP - 1), perf_mode=DR,
                    )
            nc.scalar.copy(ckvb[:rows, m, :], pa[:rows, :])
            if m < KVT:
                sqt = sq_p.tile([128, CH], bf, name="sq")
                nc.scalar.square(sqt[:], pa[:])
                nc.tensor.matmul(pss_kv[:1, :], onec_t[:], sqt[:],
                                 start=(m == 0), stop=(m == KVT - 1))
        rrb_kv = rms_scalar(pss_kv, 1.0 / KVL, eps12_t[:1, :], "kv")

        # --- folded q (bf16), rms scale fused into the eviction
        rbp_q = rms_bcast(rrb_q, "q")
        qpe_raw = qpe_p.tile([128, 2, CH], bf, name="qperaw")
        for m in range(QBT):
            whl = w8stream.tile([128, 2, KT, 128], f8, name="wqa")
            nc.sync.dma_start(out=whl[:], in_=t["wfold2"][m])
            pa = psA.tile([128, CH], f32, name="pa")
            for pi, (wi, xp) in enumerate(((0, x8), (1, x8), (0, xlo))):
                for k in range(KP):
                    nc.tensor.matmul(
                        pa[:],
                        whl[:, wi, 2 * k:2 * k + 2, :],
                        xp[:, 2 * k:2 * k + 2, :],
                        start=(pi == 0 and k == 0),
                        stop=(pi == 2 and k == KP - 1), perf_mode=DR,
                    )
            if m < HPG:
                nc.vector.tensor_tensor(qn_t[:, m, :], pa[:], rbp_q[:],
                                        op=MULT)
            else:
                nc.vector.tensor_tensor(qpe_raw[:, m - HPG, :], pa[:],
                                        rbp_q[:], op=MULT)

        # --- normalize latent in place; k_pe: dup to both halves, rope
        rbp_kv = rms_bcast(rrb_kv, "kv")
        for m in range(KVT):
            nc.vector.tensor_tensor(ckvb[:, m, :], ckvb[:, m, :], rbp_kv[:],
                                    op=MULT)
        pd = psA.tile([128, CH], f32, name="pa")
        nc.tensor.matmul(pd[:], dup_t[:], ckvb[:ROPE, KVT, :],
                         start=True, stop=True)
        dup_sb = rope_p.tile([128, CH], bf, name="rdup")
        nc.scalar.copy(dup_sb[:], pd[:])
        pp = psA.tile([128, CH], f32, name="pa")
        nc.tensor.matmul(pp[:], perm_t[:], dup_sb[:], start=True, stop=True)
        rope_combine(kpe_t[:, cs], dup_sb, pp, cs)

        # --- rope on q pe pairs
        for p in range(2):
            pp = psA.tile([128, CH], f32, name="pa")
            nc.tensor.matmul(pp[:], perm_t[:], qpe_raw[:, p, :],
                             start=True, stop=True)
            rope_combine(qpe_t[:, p, :], qpe_raw[:, p, :], pp, cs)

        # --- k embed (bf16) -> kno
        for h in range(HPG):
            pa = psA.tile([128, CH], f32, name="pa")
            for k in range(KVT):
                nc.tensor.matmul(pa[:], wemb_t[:, k, h * NOPE:(h + 1) * NOPE],
                                 ckvb[:, k, :], start=(k == 0),
                                 stop=(k == KVT - 1))
            nc.scalar.copy(kno_t[h][:, cs], pa[:])

        # --- v unembed (bf16) -> v blocks
        for h in range(HPG):
            for pi in range(QT):
                pa = psA.tile([128, CH], f32, name="pa")
                for k in range(KVT):
                    nc.tensor.matmul(
                        pa[:, :VD],
                        ckvb[:, k, pi * 128:(pi + 1) * 128],
                        wunb_t[:, k, h * VD:(h + 1) * VD],
                        start=(k == 0), stop=(k == KVT - 1),
                    )
                nc.scalar.copy(v_t[h][:, c * QT + pi, :VD], pa[:, :VD])

    def attention(c, qn_t, qpe_t):
        """Attention + o_proj for chunk c's queries (transposed scores)."""
        nb = (c + 1) * QT if causal else NBLK
        if c == 0:
            load_consts_attn()
        oh = oh_p.tile([128, HPG, CH], f8, name="oh")
        ol = oh_p.tile([128, HPG, CH], f8, name="ol")
        for h in range(HPG):
            hb = (h % 2) * 64
            psb = psb_p.tile([128, NBLK, CH], bf, name="psb")
            done = 0
            for ql in range(QT):
                nbq = c * QT + ql + 1 if causal else NBLK
                # emit only the score blocks this q-tile newly needs, so the
                # attn@V matmuls interleave with exp on the Act engine
                for bt in range(done, nbq):
                    qoff = max(0, (bt - c * QT) * 128) if causal else 0
                    w = CH - qoff
                    ps = psS.tile([128, CH], f32, name="ps")
                    nc.tensor.matmul(ps[:, :w],
                                     kno_t[h][:, bt * 128:(bt + 1) * 128],
                                     qn_t[:, h, qoff:CH],
                                     start=True, stop=False)
                    nc.tensor.matmul(ps[:, :w],
                                     kpe_t[hb:hb + 64, bt * 128:(bt + 1) * 128],
                                     qpe_t[hb:hb + 64, h // 2, qoff:CH],
                                     start=False, stop=True)
                    if causal and bt >= c * QT:
                        nc.vector.tensor_add(ps[:, :128], ps[:, :128],
                                             tri_t[:])
                    nc.scalar.activation(psb[:, bt, qoff:CH], ps[:, :w],
                                         AF.Exp, bias=m2_t[:], scale=1.0)
                done = nbq
                qsl = slice(ql * 128, (ql + 1) * 128)
                pv = psPV.tile([128, VD + 1], f32, name="pv")
                for bt in range(nbq):
                    nc.tensor.matmul(
                        pv[:, :VD + 1],
                        psb[:, bt, qsl],
                        v_t[h][:, bt, :VD + 1],
                        start=(bt == 0), stop=(bt == nbq - 1),
                    )
                rec = rec_p.tile([128, 1], f32, name="rec")
                nc.vector.reciprocal(rec[:], pv[:, VD:VD + 1])
                pvb = pvb_p.tile([128, VD], bf, name="pvb")
                nc.scalar.mul(pvb[:], pv[:, :VD], rec[:])
                pt = psT.tile([128, 128], bf, name="pt")
                nc.tensor.transpose(pt[:], pvb[:], id_t[:])
                nc.vector.tensor_copy(oh[:, h, qsl], pt[:])
                nc.vector.tensor_tensor(ol[:, h, qsl], pt[:], oh[:, h, qsl],
                                        op=mybir.AluOpType.subtract)
        # o_proj (bf16)
        for ql in range(QT):
            qsl = slice(ql * 128, (ql + 1) * 128)
            qt_g = c * QT + ql
            for nn in range(HID // 512):
                nsl = slice(nn * 512, (nn + 1) * 512)
                # ping-pong with psPV (idle once attn@V is done) to
                # double-buffer the o_proj accumulation
                if (ql * (HID // 512) + nn) % 2 == 0:
                    pw = psO.tile([128, 512], f32, name="pw")
                else:
                    pw = psPV.tile([128, 512], f32, name="pv")
                passes = ((oh, wot_t), (oh, wol_t), (ol, wot_t))
                for pi, (os_, ws_) in enumerate(passes):
                    for p in range(HPG // 2):
                        nc.tensor.matmul(
                            pw[:],
                            os_[:, 2 * p:2 * p + 2, qsl],
                            ws_[:, 2 * p:2 * p + 2, nsl],
                            start=(pi == 0 and p == 0),
                            stop=(pi == 2 and p == HPG // 2 - 1),
                            perf_mode=DR,
                        )
                stg = st_p.tile([128, 512], bf, name="stg")
                nc.vector.tensor_scalar_mul(stg[:], pw[:], float(2.0 ** -9))
                nc.sync.dma_start(
                    out=t["out"][qt_g * 128:(qt_g + 1) * 128,
                                 nn * 512:(nn + 1) * 512],
                    in_=stg[:])

    def qa8_prepass():
        """Sharded qa8 sumsq for chunks 1..3: each core computes its 3 of 12
        row-tiles, one AllReduce(add) over the batch group merges them. The
        28us modeled collective latency hides behind chunk 0's compute."""
        ploc = nc.dram_tensor("qa8_ploc", [NCH - 1, CH], mybir.dt.float32,
                              kind="Internal").ap()
        pred = nc.dram_tensor("qa8_pred", [NCH - 1, CH], mybir.dt.float32,
                              kind="Internal").ap()
        wq3 = []
        for m in range(MT // GPB):
            wcol = w8stream.tile([128, KT, 128], f8, name="wqa")
            nc.sync.dma_start(out=wcol[:], in_=t["wqas"][m])
            wq3.append(wcol)
        for ci in range(NCH - 1):
            x8c = x8_p.tile([128, KT, CH], f8, name="x8")
            nc.sync.dma_start(out=x8c[:], in_=t["x8"][ci + 1])
            pss = psS.tile([128, CH], f32, name="ps")
            for m in range(MT // GPB):
                pa = psA.tile([128, CH], f32, name="pa")
                for k in range(KP):
                    nc.tensor.matmul(
                        pa[:], wq3[m][:, 2 * k:2 * k + 2, :],
                        x8c[:, 2 * k:2 * k + 2, :],
                        start=(k == 0), stop=(k == KP - 1), perf_mode=DR,
                    )
                sqt = sq_p.tile([128, CH], bf, name="sq")
                nc.scalar.square(sqt[:], pa[:])
                nc.tensor.matmul(pss[:1, :], onec_t[:], sqt[:],
                                 start=(m == 0), stop=(m == MT // GPB - 1))
            pse = ssr_p.tile([1, CH], f32, name="ssr")
            nc.scalar.copy(pse[:], pss[:1, :])
            nc.sync.dma_start(out=ploc[ci], in_=pse[:])
        nc.gpsimd.collective_compute(
            "AllReduce", mybir.AluOpType.add,
            replica_groups=[[0, 1, 2, 3], [4, 5, 6, 7]],
            ins=[ploc[:].opt()], outs=[pred[:].opt()],
        )
        for ci in range(NCH - 1):
            ssr = ssr_p.tile([1, CH], f32, name="ssr")
            nc.sync.dma_start(out=ssr[:], in_=pred[ci])
            rrb = rms_scalar(ssr, 64.0 / QL, eps18_t[:1, :], "q")
            nc.vector.tensor_copy(pre_rrb[:1, ci + 1, :], rrb[:])

    if causal:
        qn_pool = ctx.enter_context(tc.tile_pool(name="qnc", bufs=1))
        qpe_pool = ctx.enter_context(tc.tile_pool(name="qpec", bufs=1))
        prew8 = prefetch_w8()
        xnext = load_x(0)
        for c in range(NCH):
            qn_t = qn_pool.tile([128, HPG, CH], bf, name="qnt")
            qpe_t = qpe_pool.tile([128, 2, CH], bf, name="qpet")
            phases_proj(c, qn_t, qpe_t, *xnext, pre_w8=prew8)
            if c == 0:
                # sharded sumsq for chunks 1-3; the collective latency hides
                # behind attention(0) and proj(1)
                qa8_prepass()
            if c + 1 < NCH:
                xnext = load_x(c + 1)   # prefetch during attention(c)
            attention(c, qn_t, qpe_t)
    else:
        xnext = load_x(0)
        for c in range(NCH):
            phases_proj(c, qn_all[:, c], qpe_all[:, c], *xnext)
            if c + 1 < NCH:
                xnext = load_x(c + 1)
        for c in range(NCH):
            attention(c, qn_all[:, c], qpe_all[:, c])


def _build_program(causal):
    import concourse.bass as bass
    import concourse.tile as tile
    from concourse import bacc, mybir

    dt = mybir.dt
    nc = bacc.Bacc("TRN2", target_bir_lowering=False, debug=False,
                   enable_asserts=False, num_devices=NCORES)

    def din(name, shape, dtype):
        return nc.dram_tensor(name, shape, dtype, kind="ExternalInput").ap()

    t = {
        "x8": din("x8", [NCH, 128, KT, CH], dt.float8e4),
        "xlo": din("xlo", [NCH, 128, KT, CH], dt.float8e4),
        "wqa": din("wqa", [MT, 128, KT, 128], dt.float8e4),
        "wqas": din("wqas", [MT // GPB, 128, KT, 128], dt.float8e4),
        "wfold2": din("wfold2", [QBT, 128, 2, KT, 128], dt.float8e4),
        "wkva2": din("wkva2", [KVT + 1, 128, 2, KT, 128], dt.float8e4),
        "wemb": din("wemb", [128, KVT, HPG * NOPE], dt.bfloat16),
        "wunb": din("wunb", [128, KVT, HPG * VD], dt.bfloat16),
        "wot": din("wot", [128, HPG, HID], dt.float8e4),
        "wol": din("wol", [128, HPG, HID], dt.float8e4),
        "cos": din("cos", [128, L], dt.bfloat16),
        "sin": din("sin", [128, L], dt.bfloat16),
        "perm": din("perm", [128, 128], dt.bfloat16),
        "dup": din("dup", [64, 128], dt.bfloat16),
        "ident": din("ident", [128, 128], dt.bfloat16),
        "out": nc.dram_tensor("out", [L, HID], dt.bfloat16,
                              kind="ExternalOutput").ap(),
    }
    if causal:
        t["tri"] = din("tri", [128, 128], dt.float32)

    with tile.TileContext(nc) as tc:
        with ExitStack() as ctx:
            _emit(ctx, tc, t, causal)
    nc.compile()
    return nc


def _get_program(causal):
    if causal not in _PROG_CACHE:
        _PROG_CACHE[causal] = _build_program(causal)
    return _PROG_CACHE[causal]


def _rope_tables():
    freqs = THETA ** (-np.arange(0, ROPE, 2, dtype=np.float64) / ROPE)
    th = np.arange(L, dtype=np.float64)[None, :] * freqs[:, None]  # [32, L]
    cos64 = np.repeat(np.cos(th), 2, axis=0)                       # [64, L]
    sin64 = np.repeat(np.sin(th), 2, axis=0)
    sin64[0::2] *= -1.0
    cosE = np.vstack([cos64, cos64])
    sinE = np.vstack([sin64, sin64])
    return _bf(cosE), _bf(sinE)


def _perm_matrix():
    p64 = np.zeros((64, 64), np.float32)
    for i in range(32):
        p64[2 * i + 1, 2 * i] = 1.0
        p64[2 * i, 2 * i + 1] = 1.0
    pm = np.zeros((128, 128), np.float32)
    pm[:64, :64] = p64
    pm[64:, 64:] = p64
    return _bf(pm)


def make_in_maps(x, mask, wq_a, q_a_ln_w, wq_b, wkv_a, kv_a_ln_w,
                 w_embed_q, w_unembed, wo):
    x = np.asarray(x, np.float32)
    m = np.asarray(mask).reshape(L, L)
    causal = bool(np.array_equal(m, np.tril(np.ones((L, L), bool))))
    if not causal and not m.all():
        raise NotImplementedError("only causal or all-true masks supported")

    wq_a = np.asarray(wq_a, np.float32)
    q_a_ln_w = np.asarray(q_a_ln_w, np.float32)
    wq_b = np.asarray(wq_b, np.float32)
    wkv_a = np.asarray(wkv_a, np.float32)
    kv_a_ln_w = np.asarray(kv_a_ln_w, np.float32)
    w_embed_q = np.asarray(w_embed_q, np.float32)
    w_unembed = np.asarray(w_unembed, np.float32)
    wo = np.asarray(wo, np.float32)

    # fold scalar factors + LN weights into adjacent matrices
    wqb_f = wq_b * (SQ * SCALE) * q_a_ln_w[None, :]
    kvf = SKV * kv_a_ln_w
    wemb_f = w_embed_q * kvf[None, :, None]     # [H, KVL, NOPE]
    wunb_f = w_unembed * kvf[None, None, :]     # [H, VD, KVL]
    wfold_full = wqb_f @ wq_a                   # [H*QKD, HID]

    # shared packs
    wqa_p = _f8((wq_a * float(1 << S_A)).T
                .reshape(KT, 128, MT, 128).transpose(2, 1, 0, 3))
    wkva_pad = np.zeros((KVT + 1, KT, 128, 128), np.float32)
    wkva_cols = (wkv_a * 64.0).T.reshape(KT, 128, KVL + ROPE)
    for mm in range(KVT + 1):
        rows = 128 if mm < KVT else ROPE
        wkva_pad[mm, :, :, :rows] = wkva_cols[:, :, mm * 128:mm * 128 + rows]
    wkva_pm = wkva_pad.transpose(0, 2, 1, 3)
    wkva_hi = _f8(wkva_pm)
    wkva_lo = _f8(wkva_pm - wkva_hi.astype(np.float32))
    wkva2_p = np.ascontiguousarray(np.stack([wkva_hi, wkva_lo], axis=2))
    x8_p, xlo_p = [], []
    for b in range(B):
        xpm = x[b].T.reshape(KT, 128, NCH, CH).transpose(2, 1, 0, 3)
        xh = _f8(xpm)
        x8_p.append(xh)
        xlo_p.append(_f8(xpm - xh.astype(np.float32)))
    cosE, sinE = _rope_tables()
    permM = _perm_matrix()
    ident = _bf(np.eye(128, dtype=np.float32))
    dup = np.zeros((64, 128), np.float32)
    dup[np.arange(128) % 64, np.arange(128)] = 1.0 / 64.0
    dup = _bf(dup)
    # transposed causal tri: rows = k, cols = q; allow k <= q
    tri = np.where(
        np.arange(128)[:, None] <= np.arange(128)[None, :], 0.0, -1e30
    ).astype(np.float32)

    in_maps = []
    for cidx in range(NCORES):
        b = cidx // GPB
        g = cidx % GPB
        heads = list(range(g * HPG, (g + 1) * HPG))
        # folded-q rows: nope rows of the 4 heads, then pe rows packed as two
        # 128-row head pairs
        rows = [wfold_full[h * QKD: h * QKD + NOPE] for h in heads]
        rows += [
            np.vstack([wfold_full[heads[2 * p] * QKD + NOPE:
                                  heads[2 * p] * QKD + QKD],
                       wfold_full[heads[2 * p + 1] * QKD + NOPE:
                                  heads[2 * p + 1] * QKD + QKD]])
            for p in range(2)
        ]
        wfold_r = np.vstack(rows)                            # [768, HID]
        wfold_pm = (wfold_r * 512.0).T.reshape(KT, 128, QBT, 128).transpose(2, 1, 0, 3)
        wfold_hi = _f8(wfold_pm)
        wfold_lo = _f8(wfold_pm - wfold_hi.astype(np.float32))
        wfold2_c = np.ascontiguousarray(np.stack([wfold_hi, wfold_lo], axis=2))
        wemb_c = _bf(wemb_f[heads].transpose(1, 0, 2)
                     .reshape(KVT, 128, HPG * NOPE).transpose(1, 0, 2))
        wunb_c = _bf(wunb_f[heads].transpose(2, 0, 1)
                     .reshape(KVT, 128, HPG * VD).transpose(1, 0, 2))
        wot_pm = (wo[:, g * HPG * VD:(g + 1) * HPG * VD] * 32.0).T \
            .reshape(HPG, VD, HID).transpose(1, 0, 2)
        wot_c = _f8(wot_pm)
        wot_lo = _f8(wot_pm - wot_c.astype(np.float32))
        im = {
            "x8": x8_p[b], "xlo": xlo_p[b], "wqa": wqa_p,
            "wqas": np.ascontiguousarray(
                wqa_p[g * (MT // GPB):(g + 1) * (MT // GPB)]),
            "wfold2": wfold2_c, "wkva2": wkva2_p,
            "wemb": wemb_c, "wunb": wunb_c, "wot": wot_c, "wol": wot_lo,
            "cos": cosE, "sin": sinE, "perm": permM, "dup": dup,
            "ident": ident,
        }
        if causal:
            im["tri"] = tri
        in_maps.append(im)
    return in_maps, causal


def assemble(results):
    out = np.zeros((B, L, HID), np.float32)
    for c in range(NCORES):
        out[c // GPB] += results[c]["out"].astype(np.float32)
    return out


def kernel(**inputs):
    from concourse.bass_utils import run_bass_kernel_spmd

    in_maps, causal = make_in_maps(**inputs)
    nc = _get_program(causal)
    res = run_bass_kernel_spmd(nc, in_maps, list(range(NCORES)))
    return assemble(res.results)
